# revision 1
# baseline (speedup 1.0000x reference)
"""LogitLinear Trainium2 kernel: softmax-moment weights + dual GEMM.

out[n, 0, o] = sum_i mean(W_logits[:, o, i]) * x[n, i]   + mean(b_logits[:, o])
out[n, 1, o] = sum_i var(W_logits[:, o, i])  * x[n, i]^2 + var(b_logits[:, o])

p = softmax(logits over D=3 values [-1, 0, 1]); mean = p2 - p0,
E[w^2] = p0 + p2, var = E[w^2] - mean^2.

Sharding: out_feat split across 8 cores (512 each); x replicated.
Host pre-transposes W (i-major) and x (x^T, bf16) so both GEMM operands
load with the contraction dim on partitions using contiguous DMA.
"""

import numpy as np
import ml_dtypes

N, IN, OUT, D = 2048, 4096, 4096, 3
NCORES = 8
OS = OUT // NCORES  # 512 out-features per core
KB = IN // 128      # 32 contraction blocks
PAIR = 2            # kb processed per moment step
KQ = KB // PAIR
NT = N // 128       # 16 n-tiles
WAVE = 4            # n-tiles per PSUM wave
NWAVES = NT // WAVE
WS = WAVE * 128     # 512 columns per wave
SKEW = 1            # kq-skew for the var-weight chain (m2 on ACT)

_CACHED_NC = None


def _build():
    global _CACHED_NC
    if _CACHED_NC is not None:
        return _CACHED_NC
    import concourse.bass as bass
    import concourse.bacc as bacc
    import concourse.mybir as mybir
    import concourse.tile as tile

    dt = mybir.dt
    f32, bf16 = dt.float32, dt.bfloat16
    Exp = mybir.ActivationFunctionType.Exp
    Square = mybir.ActivationFunctionType.Square

    nc = bacc.Bacc("TRN2", debug=False, num_devices=NCORES)
    xt = nc.dram_tensor("xt", [IN, N], bf16, kind="ExternalInput")
    wt = nc.dram_tensor("wt", [D, IN, OS], f32, kind="ExternalInput")
    bl = nc.dram_tensor("bl", [D, OS], f32, kind="ExternalInput")
    out = nc.dram_tensor("out", [N, 2, OS], f32, kind="ExternalOutput")

    # kb-pair views: partition = i within block, p2 = which kb of the pair
    xt_ap = xt.ap().rearrange("(kq p2 p) n -> kq p p2 n", p=128, p2=PAIR)
    wt_ap = wt.ap().rearrange("d (kq p2 p) o -> kq p d p2 o", p=128, p2=PAIR)
    out_ap = out.ap().rearrange("(nt p) m o -> nt p m o", p=128)

    with tile.TileContext(nc) as tc:
        with (
            tc.tile_pool(name="wres", bufs=1) as wres,
            tc.tile_pool(name="ld", bufs=2) as ld,
            tc.tile_pool(name="mt", bufs=2) as mt,
            tc.tile_pool(name="xs", bufs=4) as xs,
            tc.tile_pool(name="st", bufs=3) as st,
            tc.tile_pool(name="bias", bufs=1) as bias,
            tc.tile_pool(name="ps", bufs=8, space="PSUM") as ps,
        ):
            wTm = wres.tile([128, KB, OS], bf16, tag="wTm")
            wTv = wres.tile([128, KB, OS], bf16, tag="wTv")

            # warm the ACT exp table set before the first real exp needs it
            warm = wres.tile([1, 8], f32, tag="warm")
            nc.vector.memset(warm, 0.0)
            nc.scalar.activation(out=warm, in_=warm, func=Exp)

            s1_t = [None] * KQ
            rb_t = [None] * KQ

            def emit_moments_front(kq):
                lt = ld.tile([128, D, PAIR, OS], f32, tag="lt")
                for di in range(D):
                    nc.sync.dma_start(out=lt[:, di], in_=wt_ap[kq][:, di])
                e = mt.tile([128, D, PAIR, OS], bf16, tag="e")
                nc.scalar.activation(out=e, in_=lt, func=Exp)
                s1 = mt.tile([128, PAIR, OS], bf16, tag="s1", bufs=SKEW + 2)
                nc.gpsimd.tensor_add(s1, e[:, 2], e[:, 0])
                s = mt.tile([128, PAIR, OS], f32, tag="s")
                nc.gpsimd.tensor_add(s, s1, e[:, 1])
                r = mt.tile([128, PAIR, OS], f32, tag="r")
                nc.vector.reciprocal_approx_fast(out=r, in_=s)
                rb = mt.tile([128, PAIR, OS], bf16, tag="rb", bufs=SKEW + 2)
                nc.vector.tensor_copy(rb, r)
                a = mt.tile([128, PAIR, OS], bf16, tag="a")
                nc.vector.tensor_sub(a, e[:, 2], e[:, 0])
                nc.vector.tensor_mul(
                    wTm[:, PAIR * kq : PAIR * (kq + 1), :], a, rb
                )
                s1_t[kq], rb_t[kq] = s1, rb

            def emit_moments_back(kq):
                msl = wTm[:, PAIR * kq : PAIR * (kq + 1), :]
                m2 = mt.tile([128, PAIR, OS], bf16, tag="m2")
                nc.scalar.activation(out=m2, in_=msl, func=Square)
                sq = mt.tile([128, PAIR, OS], bf16, tag="sq")
                nc.vector.tensor_mul(sq, s1_t[kq], rb_t[kq])
                nc.vector.tensor_sub(
                    wTv[:, PAIR * kq : PAIR * (kq + 1), :], sq, m2
                )
                s1_t[kq] = rb_t[kq] = None

            def emit_bias():
                bl_ap = bl.ap()
                bl_bcast = bass.AP(
                    tensor=bl_ap.tensor,
                    offset=bl_ap.offset,
                    ap=[[0, 128]] + [list(p) for p in bl_ap.ap],
                )
                bl_t = ld.tile([128, D, OS], f32, tag="lt")
                nc.gpsimd.dma_start(out=bl_t, in_=bl_bcast)
                eb = mt.tile([128, D, OS], f32, tag="e")
                nc.scalar.activation(out=eb, in_=bl_t, func=Exp)
                bs1 = mt.tile([128, OS], f32, tag="s1", bufs=SKEW + 2)
                nc.vector.tensor_add(bs1, eb[:, 2, :], eb[:, 0, :])
                bs = mt.tile([128, OS], f32, tag="s")
                nc.vector.tensor_add(bs, bs1, eb[:, 1, :])
                br = mt.tile([128, OS], f32, tag="r")
                nc.vector.reciprocal_approx_fast(out=br, in_=bs)
                bA = mt.tile([128, OS], f32, tag="a")
                nc.vector.tensor_sub(bA, eb[:, 2, :], eb[:, 0, :])
                bmean = bias.tile([128, OS], f32, tag="bmean")
                nc.vector.tensor_mul(bmean, bA, br)
                bm2 = mt.tile([128, OS], f32, tag="m2")
                nc.vector.tensor_mul(bm2, bmean, bmean)
                bsq = mt.tile([128, OS], f32, tag="sq")
                nc.vector.tensor_mul(bsq, bs1, br)
                bvar = bias.tile([128, OS], f32, tag="bvar")
                nc.vector.tensor_sub(bvar, bsq, bm2)
                return bmean, bvar

            bmean = bvar = None
            for w in range(NWAVES):
                psm = [
                    ps.tile([128, OS], f32, tag="ps", name=f"psm{w}_{j}")
                    for j in range(WAVE)
                ]
                psv = [
                    ps.tile([128, OS], f32, tag="ps", name=f"psv{w}_{j}")
                    for j in range(WAVE)
                ]
                first = w == 0
                xx_slabs = {}

                def emit_var_mms(kq):
                    for kbi in range(PAIR):
                        kb = PAIR * kq + kbi
                        for j in range(WAVE):
                            nc.tensor.matmul(
                                psv[j],
                                lhsT=xx_slabs[kq][
                                    :, kbi, j * 128 : (j + 1) * 128
                                ],
                                rhs=wTv[:, kb, :],
                                start=(kb == 0),
                                stop=(kb == KB - 1),
                            )
                    del xx_slabs[kq]

                for kq in range(KQ):
                    if first:
                        emit_moments_front(kq)
                        if kq >= SKEW:
                            emit_moments_back(kq - SKEW)
                    xsl = xs.tile([128, PAIR, WS], bf16, tag="xsl")
                    nc.sync.dma_start(
                        out=xsl, in_=xt_ap[kq][:, :, w * WS : (w + 1) * WS]
                    )
                    xxl = xs.tile(
                        [128, PAIR, WS], bf16, tag="xxl", bufs=SKEW + 3
                    )
                    nc.vector.tensor_mul(xxl, xsl, xsl)
                    xx_slabs[kq] = xxl
                    for kbi in range(PAIR):
                        kb = PAIR * kq + kbi
                        for j in range(WAVE):
                            nc.tensor.matmul(
                                psm[j],
                                lhsT=xsl[:, kbi, j * 128 : (j + 1) * 128],
                                rhs=wTm[:, kb, :],
                                start=(kb == 0),
                                stop=(kb == KB - 1),
                            )
                    kqv = kq - SKEW if first else kq
                    if kqv >= 0:
                        emit_var_mms(kqv)
                if first:
                    bmean, bvar = emit_bias()
                    for kq in range(KQ - SKEW, KQ):
                        emit_moments_back(kq)
                        emit_var_mms(kq)
                for j in range(WAVE):
                    stg = st.tile([128, 2, OS], f32, tag="stg")
                    nc.vector.tensor_add(stg[:, 0, :], psm[j], bmean)
                    nc.vector.tensor_add(stg[:, 1, :], psv[j], bvar)
                    nc.sync.dma_start(out=out_ap[w * WAVE + j], in_=stg)

    nc.compile()
    _CACHED_NC = nc
    return nc


def kernel(x, W_logits, b_logits):
    from concourse import bass_utils

    nc = _build()
    xt_b = np.ascontiguousarray(x.T).astype(ml_dtypes.bfloat16)
    in_maps = []
    for c in range(NCORES):
        sl = slice(c * OS, (c + 1) * OS)
        wt_c = np.ascontiguousarray(W_logits[:, sl, :].transpose(0, 2, 1))
        bl_c = np.ascontiguousarray(b_logits[:, sl, 0])
        in_maps.append({"xt": xt_b, "wt": wt_c, "bl": bl_c})
    res = bass_utils.run_bass_kernel_spmd(
        nc, in_maps, core_ids=list(range(NCORES))
    )
    full = np.empty((N, 2, OUT), dtype=np.float32)
    for c in range(NCORES):
        full[:, :, c * OS : (c + 1) * OS] = res.results[c]["out"]
    return full



# revision 6
# speedup vs baseline: 1.2123x; 1.2123x over previous
"""LogitLinear Trainium2 kernel: softmax-moment weights + dual fp8 GEMM.

out[n, 0, o] = sum_i mean(W_logits[:, o, i]) * x[n, i]   + mean(b_logits[:, o])
out[n, 1, o] = sum_i var(W_logits[:, o, i])  * x[n, i]^2 + var(b_logits[:, o])

Per (o, i): with u = e^{l0-l1}, v = e^{l2-l1}, r = 1/(1+u+v):
  mean = (v-u)*r,  E[w^2] = (u+v)*r = 1-r,  var = (1-r) - mean^2.
Moment weights and x are quantized to fp8(e4m3) and both GEMMs run as
DoubleRow fp8 matmuls (2 contraction blocks per MM, 2x PE throughput).
r comes from a fused custom-DVE op (bitwise-not seed + 1 Newton step).

Sharding: out_feat split across 8 cores (512 each); x replicated.
Host pre-transposes W logits (i-major, bf16) and x (x^T, fp8).
"""

import numpy as np
import ml_dtypes

N, IN, OUT, D = 2048, 4096, 4096, 3
NCORES = 8
OS = OUT // NCORES  # 512 out-features per core
KB = IN // 128      # 32 contraction blocks
PAIR = 2            # kb per DoubleRow matmul / moment step
KQ = KB // PAIR     # 16
NT = N // 128       # 16 n-tiles
WAVE = 4            # n-tiles per PSUM wave
NWAVES = NT // WAVE
WS = WAVE * 128     # 512 n-columns per wave
SKEW = 2            # kq-skew of var matmuls behind the moment pipeline

RECIP_C0, RECIP_C1 = -0.23549792, 2.0017324

_CACHED_NC = None


def _register_ll_ops():
    """Register fused DVE ops for the moment pipeline (idempotent)."""
    import concourse.dve_ops as dvo
    from concourse.dve_spec import (
        Spec, Src0, Src1, C0, C1, One, Bin, AluOp, lower, _has_src1, sq,
    )
    from concourse.dve_uop import DveOpSpec

    def _recip1(x):
        # bitwise-not exponent-flip seed + Chebyshev scale + 1 Newton pass
        n = Bin(AluOp.BITWISE_NOT, x, x)
        y0 = n * C0
        return y0 * (C1 - x * y0)

    def _np_recip1(x, c0, c1):
        xf = np.ascontiguousarray(x, dtype=np.float32)
        nx = (~xf.view(np.int32)).view(np.float32)
        y0 = nx * c0
        return y0 * (c1 - xf * y0)

    specs = {
        # out = recip1(in0 + 1)
        "LL_RECIP": Spec(
            body=_recip1(Src0 + One),
            reference=lambda in0, in1, s0, s1, imm2: _np_recip1(
                np.asarray(in0, np.float32) + 1.0, s0, s1
            ),
        ),
        # out = (1 - in0) - (in1*in0)^2   [in0=r, in1=a]
        "LL_VARR": Spec(
            body=(One - Src0) - sq(Src1 * Src0),
            reference=lambda in0, in1, s0, s1, imm2: (
                (1.0 - np.asarray(in0, np.float32))
                - (np.asarray(in1, np.float32) * np.asarray(in0, np.float32)) ** 2
            ),
        ),
    }
    ops = {}
    by_name = {op.name: op for op in dvo.OPS}
    for name, spec in specs.items():
        if name in by_name:
            ops[name] = by_name[name]
            continue
        row = dvo._CUSTOM_DVE_ROW_BASE + len(dvo.OPS)
        shas = {}
        for ver in ("v3", "v4"):
            uops = lower(spec, ver=ver)
            shas[ver] = DveOpSpec(
                name=name, opcode=row, uops=uops, rd1_en=_has_src1(spec)
            ).sha(ver)
        op = dvo.DveOp(name, spec, subdim=False, uops_sha=shas)
        dvo.OPS.append(op)
        dvo.CUSTOM_DVE_SPECS[name] = spec
        dvo._SUB_OPCODE_FOR_NAME[name] = row
        ops[name] = op
    return ops


def _build():
    global _CACHED_NC
    if _CACHED_NC is not None:
        return _CACHED_NC
    import concourse.bass as bass
    import concourse.bacc as bacc
    import concourse.mybir as mybir
    import concourse.tile as tile

    ops = _register_ll_ops()
    LL_RECIP, LL_VARR = ops["LL_RECIP"], ops["LL_VARR"]

    dt = mybir.dt
    f32, bf16, f8 = dt.float32, dt.bfloat16, dt.float8e4
    Exp = mybir.ActivationFunctionType.Exp
    Copy = mybir.ActivationFunctionType.Copy
    Square = mybir.ActivationFunctionType.Square
    DR = mybir.MatmulPerfMode.DoubleRow

    nc = bacc.Bacc("TRN2", debug=False, num_devices=NCORES)
    xt = nc.dram_tensor("xt", [IN, N], f8, kind="ExternalInput")
    wt = nc.dram_tensor("wt", [D, IN, OS], bf16, kind="ExternalInput")
    bl = nc.dram_tensor("bl", [D, OS], f32, kind="ExternalInput")
    out = nc.dram_tensor("out", [N, 2, OS], f32, kind="ExternalOutput")

    # x^T: partition = i within 128-block, free = [kb, n]
    xt_ap = xt.ap().rearrange("(kb p) n -> p kb n", p=128)
    # logits per kq: [p, d, pair, OS]
    wt_ap = wt.ap().rearrange("d (kq p2 p) o -> kq p d p2 o", p=128, p2=PAIR)
    out_ap = out.ap().rearrange("(nt p) m o -> nt p m o", p=128)

    with tile.TileContext(nc) as tc:
        with (
            tc.tile_pool(name="wres", bufs=1) as wres,
            tc.tile_pool(name="ld", bufs=2) as ld,
            tc.tile_pool(name="mt", bufs=2) as mt,
            tc.tile_pool(name="xs", bufs=2) as xs,
            tc.tile_pool(name="st", bufs=3) as st,
            tc.tile_pool(name="bias", bufs=1) as bias,
            tc.tile_pool(name="ps", bufs=8, space="PSUM") as ps,
        ):
            wTm = wres.tile([128, KB, OS], f8, tag="wTm")
            wTv = wres.tile([128, KB, OS], f8, tag="wTv")
            xfull = wres.tile([128, KB, N], f8, tag="xfull")
            ones = wres.tile([1, 128], bf16, tag="ones")
            nc.vector.memset(ones, 1.0)

            # x^T loaded once, in per-wave column slabs
            for w in range(NWAVES):
                nc.sync.dma_start(
                    out=xfull[:, :, w * WS : (w + 1) * WS],
                    in_=xt_ap[:, :, w * WS : (w + 1) * WS],
                )

            # warm the ACT exp table before the first real exp
            warm = wres.tile([1, 8], f32, tag="warm")
            nc.vector.memset(warm, 0.0)
            nc.scalar.activation(out=warm, in_=warm, func=Exp)

            def emit_bias():
                bl_ap = bl.ap()
                bl_bcast = bass.AP(
                    tensor=bl_ap.tensor,
                    offset=bl_ap.offset,
                    ap=[[0, 128]] + [list(p) for p in bl_ap.ap],
                )
                blt = bias.tile([128, D, OS], f32, tag="blt")
                nc.gpsimd.dma_start(out=blt, in_=bl_bcast)
                bdd = bias.tile([128, 2, OS], bf16, tag="bdd")
                nc.vector.tensor_sub(bdd[:, 0], blt[:, 0], blt[:, 1])
                nc.vector.tensor_sub(bdd[:, 1], blt[:, 2], blt[:, 1])
                bee = bias.tile([128, 2, OS], bf16, tag="bee")
                nc.scalar.activation(out=bee, in_=bdd, func=Exp)
                ba = bias.tile([128, OS], bf16, tag="ba")
                nc.vector.tensor_sub(ba, bee[:, 1], bee[:, 0])
                bs = bias.tile([128, OS], bf16, tag="bs")
                nc.vector.tensor_add(bs, bee[:, 1], bee[:, 0])
                br = bias.tile([128, OS], bf16, tag="br")
                nc.vector._custom_dve(
                    LL_RECIP, out=br, in0=bs, s0=RECIP_C0, s1=RECIP_C1,
                )
                bmean = bias.tile([128, OS], bf16, tag="bmean")
                nc.vector.tensor_mul(bmean, ba, br)
                bvar = bias.tile([128, OS], bf16, tag="bvar")
                nc.vector._custom_dve(LL_VARR, out=bvar, in0=br, in1=ba)
                return bmean, bvar

            bmean, bvar = emit_bias()

            def emit_moments(kq):
                lt = ld.tile([128, D, PAIR, OS], bf16, tag="lt")
                for di in range(D):
                    nc.sync.dma_start(out=lt[:, di], in_=wt_ap[kq][:, di])
                dd = mt.tile([128, 2, PAIR, OS], bf16, tag="dd")
                nc.vector.tensor_sub(dd[:, 0], lt[:, 0], lt[:, 1])
                nc.gpsimd.tensor_sub(dd[:, 1], lt[:, 2], lt[:, 1])
                ee = mt.tile([128, 2, PAIR, OS], bf16, tag="ee")
                nc.scalar.activation(out=ee, in_=dd, func=Exp)
                a = mt.tile([128, PAIR, OS], bf16, tag="a", bufs=SKEW + 2)
                nc.gpsimd.tensor_sub(a, ee[:, 1], ee[:, 0])
                s = mt.tile([128, PAIR, OS], bf16, tag="s")
                nc.vector.tensor_add(s, ee[:, 1], ee[:, 0])
                r = mt.tile([128, PAIR, OS], bf16, tag="r", bufs=SKEW + 2)
                nc.vector._custom_dve(
                    LL_RECIP, out=r, in0=s, s0=RECIP_C0, s1=RECIP_C1,
                )
                nc.vector.tensor_mul(
                    wTm[:, PAIR * kq : PAIR * (kq + 1), :], a, r
                )
                return a, r

            def emit_var_weights(kq, a, r):
                nc.vector._custom_dve(
                    LL_VARR,
                    out=wTv[:, PAIR * kq : PAIR * (kq + 1), :],
                    in0=r, in1=a,
                )

            ar_t = [None] * KQ

            for w in range(NWAVES):
                psm = [
                    ps.tile([128, OS], f32, tag="ps", name=f"psm{w}_{j}")
                    for j in range(WAVE)
                ]
                psv = [
                    ps.tile([128, OS], f32, tag="ps", name=f"psv{w}_{j}")
                    for j in range(WAVE)
                ]
                first = w == 0

                # x^2 slab for this wave (fp8): waves 0,2 on ACT; 1,3 on DVE
                xsl = xfull[:, :, w * WS : (w + 1) * WS]
                xx = xs.tile([128, KB, WS], f8, tag="xx")
                if w % 2 == 0:
                    nc.scalar.activation(out=xx, in_=xsl, func=Square)
                else:
                    nc.vector.tensor_mul(xx, xsl, xsl)

                def emit_mean_mms(kq):
                    for j in range(WAVE):
                        nc.tensor.matmul(
                            psm[j],
                            lhsT=xsl[:, PAIR * kq : PAIR * (kq + 1),
                                     j * 128 : (j + 1) * 128],
                            rhs=wTm[:, PAIR * kq : PAIR * (kq + 1), :],
                            start=(kq == 0),
                            stop=(kq == KQ - 1),
                            perf_mode=DR,
                        )

                def emit_var_mms(kq):
                    for j in range(WAVE):
                        nc.tensor.matmul(
                            psv[j],
                            lhsT=xx[:, PAIR * kq : PAIR * (kq + 1),
                                    j * 128 : (j + 1) * 128],
                            rhs=wTv[:, PAIR * kq : PAIR * (kq + 1), :],
                            start=(kq == 0),
                            stop=False,
                            perf_mode=DR,
                        )

                for kq in range(KQ):
                    if first:
                        ar_t[kq] = emit_moments(kq)
                    emit_mean_mms(kq)
                    kqv = kq - SKEW if first else kq
                    if kqv >= 0:
                        if first:
                            emit_var_weights(kqv, *ar_t[kqv])
                            ar_t[kqv] = None
                        emit_var_mms(kqv)
                if first:
                    for kq in range(KQ - SKEW, KQ):
                        emit_var_weights(kq, *ar_t[kq])
                        ar_t[kq] = None
                        emit_var_mms(kq)

                for j in range(WAVE):
                    # bias for the var channel via K=1 matmul (closes group)
                    nc.tensor.matmul(
                        psv[j],
                        lhsT=ones,
                        rhs=bvar[0:1, :],
                        start=False,
                        stop=True,
                    )
                    stg = st.tile([128, 2, OS], f32, tag="stg")
                    nc.vector.tensor_add(stg[:, 0, :], psm[j], bmean)
                    nc.scalar.activation(out=stg[:, 1, :], in_=psv[j], func=Copy)
                    nc.sync.dma_start(out=out_ap[w * WAVE + j], in_=stg)

    nc.compile()
    _CACHED_NC = nc
    return nc


def _prep_inputs(x, W_logits, b_logits):
    f8np = ml_dtypes.float8_e4m3
    bf16np = ml_dtypes.bfloat16
    xt_8 = np.ascontiguousarray(x.T).astype(f8np)
    in_maps = []
    for c in range(NCORES):
        sl = slice(c * OS, (c + 1) * OS)
        wt_c = np.ascontiguousarray(
            W_logits[:, sl, :].transpose(0, 2, 1)
        ).astype(bf16np)
        bl_c = np.ascontiguousarray(b_logits[:, sl, 0]).astype(np.float32)
        in_maps.append({"xt": xt_8, "wt": wt_c, "bl": bl_c})
    return in_maps


def kernel(x, W_logits, b_logits):
    from concourse import bass_utils

    nc = _build()
    in_maps = _prep_inputs(x, W_logits, b_logits)
    res = bass_utils.run_bass_kernel_spmd(
        nc, in_maps, core_ids=list(range(NCORES))
    )
    full = np.empty((N, 2, OUT), dtype=np.float32)
    for c in range(NCORES):
        full[:, :, c * OS : (c + 1) * OS] = res.results[c]["out"]
    return full


# revision 7
# speedup vs baseline: 1.8364x; 1.5147x over previous
"""LogitLinear Trainium2 kernel: softmax-moment weights + dual fp8 GEMM.

out[n, 0, o] = sum_i mean(W_logits[:, o, i]) * x[n, i]   + mean(b_logits[:, o])
out[n, 1, o] = sum_i var(W_logits[:, o, i])  * x[n, i]^2 + var(b_logits[:, o])

Per (o, i): with u = e^{l0-l1}, v = e^{l2-l1}, r = 1/(1+u+v):
  mean = (v-u)*r,  E[w^2] = (u+v)*r = 1-r,  var = (1-r) - mean^2.
Moment weights, x and x^2 are fp8(e4m3); both GEMMs are DoubleRow fp8
matmuls (K=256 per MM). r comes from a fused custom-DVE op
(bitwise-not seed + 1 Newton step) applied directly to (u, v).

Sharding: out_feat split across 8 cores (512 each); x replicated.
Host prep: x^T and (x^T)^2 in fp8, logit diffs (l0-l1, l2-l1) in bf16.
"""

import numpy as np
import ml_dtypes

N, IN, OUT, D = 2048, 4096, 4096, 3
NCORES = 8
OS = OUT // NCORES  # 512 out-features per core
KB = IN // 128      # 32 contraction blocks
PAIR = 2            # kb per DoubleRow matmul
KQ = KB // PAIR     # 16 matmul steps over K
CH = 4              # kb per moment-pipeline chunk
NCH = KB // CH      # 8 chunks
NT = N // 128       # 16 n-tiles
WAVE = 4            # n-tiles per PSUM wave
NWAVES = NT // WAVE
WS = WAVE * 128     # 512 n-columns per wave
SKEW = 1            # chunk-skew of var matmuls behind the moment pipeline

RECIP_C0, RECIP_C1 = -0.23549792, 2.0017324

_CACHED_NC = None


def _register_ll_ops():
    """Register fused DVE ops for the moment pipeline (idempotent)."""
    import concourse.dve_ops as dvo
    from concourse.dve_spec import (
        Spec, Src0, Src1, C0, C1, One, Bin, AluOp, lower, _has_src1, sq,
    )
    from concourse.dve_uop import DveOpSpec

    def _recip1(x):
        # bitwise-not exponent-flip seed + Chebyshev scale + 1 Newton pass
        n = Bin(AluOp.BITWISE_NOT, x, x)
        y0 = n * C0
        return y0 * (C1 - x * y0)

    def _np_recip1(x, c0, c1):
        xf = np.ascontiguousarray(x, dtype=np.float32)
        nx = (~xf.view(np.int32)).view(np.float32)
        y0 = nx * c0
        return y0 * (c1 - xf * y0)

    specs = {
        # out = recip1(1 + in0 + in1)
        "LL_RECIPUV": Spec(
            body=_recip1(One + Src0 + Src1),
            reference=lambda in0, in1, s0, s1, imm2: _np_recip1(
                1.0 + np.asarray(in0, np.float32) + np.asarray(in1, np.float32),
                s0, s1,
            ),
        ),
        # out = (1 - in0) - (in1*in0)^2   [in0=r, in1=a]
        "LL_VARR": Spec(
            body=(One - Src0) - sq(Src1 * Src0),
            reference=lambda in0, in1, s0, s1, imm2: (
                (1.0 - np.asarray(in0, np.float32))
                - (np.asarray(in1, np.float32) * np.asarray(in0, np.float32)) ** 2
            ),
        ),
    }
    ops = {}
    by_name = {op.name: op for op in dvo.OPS}
    for name, spec in specs.items():
        if name in by_name:
            ops[name] = by_name[name]
            continue
        row = dvo._CUSTOM_DVE_ROW_BASE + len(dvo.OPS)
        shas = {}
        for ver in ("v3", "v4"):
            uops = lower(spec, ver=ver)
            shas[ver] = DveOpSpec(
                name=name, opcode=row, uops=uops, rd1_en=_has_src1(spec)
            ).sha(ver)
        op = dvo.DveOp(name, spec, subdim=False, uops_sha=shas)
        dvo.OPS.append(op)
        dvo.CUSTOM_DVE_SPECS[name] = spec
        dvo._SUB_OPCODE_FOR_NAME[name] = row
        ops[name] = op
    return ops


def _build():
    global _CACHED_NC
    if _CACHED_NC is not None:
        return _CACHED_NC
    import concourse.bass as bass
    import concourse.bacc as bacc
    import concourse.mybir as mybir
    import concourse.tile as tile

    ops = _register_ll_ops()
    LL_RECIPUV, LL_VARR = ops["LL_RECIPUV"], ops["LL_VARR"]

    dt = mybir.dt
    f32, bf16, f8 = dt.float32, dt.bfloat16, dt.float8e4
    Exp = mybir.ActivationFunctionType.Exp
    Copy = mybir.ActivationFunctionType.Copy
    DR = mybir.MatmulPerfMode.DoubleRow

    nc = bacc.Bacc("TRN2", debug=False, num_devices=NCORES)
    xt = nc.dram_tensor("xt", [IN, N], f8, kind="ExternalInput")
    xxt = nc.dram_tensor("xxt", [IN, N], f8, kind="ExternalInput")
    wd = nc.dram_tensor("wd", [2, IN, OS], bf16, kind="ExternalInput")
    bd = nc.dram_tensor("bd", [2, OS], f32, kind="ExternalInput")
    out = nc.dram_tensor("out", [N, 2, OS], f32, kind="ExternalOutput")

    # x^T / (x^T)^2: partition = i within 128-block, free = [kb, n]
    xt_ap = xt.ap().rearrange("(kb p) n -> p kb n", p=128)
    xxt_ap = xxt.ap().rearrange("(kb p) n -> p kb n", p=128)
    # logit diffs per chunk: [p, e, 4, OS]
    wd_ap = wd.ap().rearrange("e (ch p4 p) o -> ch p e p4 o", p=128, p4=CH)
    out_ap = out.ap().rearrange("(nt p) m o -> nt p m o", p=128)

    with tile.TileContext(nc) as tc:
        with (
            tc.tile_pool(name="wres", bufs=1) as wres,
            tc.tile_pool(name="ld", bufs=3) as ld,
            tc.tile_pool(name="mt", bufs=2) as mt,
            tc.tile_pool(name="xs", bufs=2) as xs,
            tc.tile_pool(name="st", bufs=3) as st,
            tc.tile_pool(name="bias", bufs=1) as bias,
            tc.tile_pool(name="ps", bufs=8, space="PSUM") as ps,
        ):
            wTm = wres.tile([128, KB, OS], f8, tag="wTm")
            wTv = wres.tile([128, KB, OS], f8, tag="wTv")
            ones = wres.tile([1, 128], bf16, tag="ones")
            nc.vector.memset(ones, 1.0)

            # warm the ACT exp table before the first real exp
            warm = wres.tile([1, 8], f32, tag="warm")
            nc.vector.memset(warm, 0.0)
            nc.scalar.activation(out=warm, in_=warm, func=Exp)

            def emit_bias():
                bd_ap = bd.ap()
                bd_bcast = bass.AP(
                    tensor=bd_ap.tensor,
                    offset=bd_ap.offset,
                    ap=[[0, 128]] + [list(p) for p in bd_ap.ap],
                )
                bdt = bias.tile([128, 2, OS], f32, tag="bdt")
                nc.gpsimd.dma_start(out=bdt, in_=bd_bcast)
                bee = bias.tile([128, 2, OS], bf16, tag="bee")
                nc.scalar.activation(out=bee, in_=bdt, func=Exp)
                ba = bias.tile([128, OS], bf16, tag="ba")
                nc.vector.tensor_sub(ba, bee[:, 1], bee[:, 0])
                br = bias.tile([128, OS], bf16, tag="br")
                nc.vector._custom_dve(
                    LL_RECIPUV, out=br, in0=bee[:, 0], in1=bee[:, 1],
                    s0=RECIP_C0, s1=RECIP_C1,
                )
                bmean = bias.tile([128, OS], bf16, tag="bmean")
                nc.vector.tensor_mul(bmean, ba, br)
                bvar = bias.tile([128, OS], bf16, tag="bvar")
                nc.vector._custom_dve(LL_VARR, out=bvar, in0=br, in1=ba)
                return bmean, bvar

            bmean, bvar = emit_bias()

            def emit_moments(ch):
                lt = ld.tile([128, 2, CH, OS], bf16, tag="lt")
                for e in range(2):
                    nc.gpsimd.dma_start(out=lt[:, e], in_=wd_ap[ch][:, e])
                ee = mt.tile([128, 2, CH, OS], bf16, tag="ee")
                nc.scalar.activation(out=ee, in_=lt, func=Exp)
                a = mt.tile([128, CH, OS], bf16, tag="a", bufs=SKEW + 2)
                nc.vector.tensor_sub(a, ee[:, 1], ee[:, 0])
                r = mt.tile([128, CH, OS], bf16, tag="r", bufs=SKEW + 2)
                nc.vector._custom_dve(
                    LL_RECIPUV, out=r, in0=ee[:, 0], in1=ee[:, 1],
                    s0=RECIP_C0, s1=RECIP_C1,
                )
                nc.vector.tensor_mul(wTm[:, CH * ch : CH * (ch + 1), :], a, r)
                return a, r

            def emit_var_weights(ch, a, r):
                nc.vector._custom_dve(
                    LL_VARR,
                    out=wTv[:, CH * ch : CH * (ch + 1), :],
                    in0=r, in1=a,
                )

            ar_t = [None] * NCH

            for w in range(NWAVES):
                psm = [
                    ps.tile([128, OS], f32, tag="ps", name=f"psm{w}_{j}")
                    for j in range(WAVE)
                ]
                psv = [
                    ps.tile([128, OS], f32, tag="ps", name=f"psv{w}_{j}")
                    for j in range(WAVE)
                ]
                first = w == 0

                xsl = xs.tile([128, KB, WS], f8, tag="xsl")
                nc.sync.dma_start(out=xsl, in_=xt_ap[:, :, w * WS : (w + 1) * WS])
                xxl = xs.tile([128, KB, WS], f8, tag="xxl")
                nc.sync.dma_start(out=xxl, in_=xxt_ap[:, :, w * WS : (w + 1) * WS])

                def emit_mean_mms(kq):
                    for j in range(WAVE):
                        nc.tensor.matmul(
                            psm[j],
                            lhsT=xsl[:, PAIR * kq : PAIR * (kq + 1),
                                     j * 128 : (j + 1) * 128],
                            rhs=wTm[:, PAIR * kq : PAIR * (kq + 1), :],
                            start=(kq == 0),
                            stop=(kq == KQ - 1),
                            perf_mode=DR,
                        )

                def emit_var_mms(kq):
                    for j in range(WAVE):
                        nc.tensor.matmul(
                            psv[j],
                            lhsT=xxl[:, PAIR * kq : PAIR * (kq + 1),
                                     j * 128 : (j + 1) * 128],
                            rhs=wTv[:, PAIR * kq : PAIR * (kq + 1), :],
                            start=(kq == 0),
                            stop=False,
                            perf_mode=DR,
                        )

                for ch in range(NCH):
                    if first:
                        ar_t[ch] = emit_moments(ch)
                    for kq in (2 * ch, 2 * ch + 1):
                        emit_mean_mms(kq)
                    chv = ch - SKEW if first else ch
                    if chv >= 0:
                        if first:
                            emit_var_weights(chv, *ar_t[chv])
                            ar_t[chv] = None
                        for kq in (2 * chv, 2 * chv + 1):
                            emit_var_mms(kq)
                if first:
                    for ch in range(NCH - SKEW, NCH):
                        emit_var_weights(ch, *ar_t[ch])
                        ar_t[ch] = None
                        for kq in (2 * ch, 2 * ch + 1):
                            emit_var_mms(kq)

                for j in range(WAVE):
                    # bias for the var channel via K=1 matmul (closes group)
                    nc.tensor.matmul(
                        psv[j],
                        lhsT=ones,
                        rhs=bvar[0:1, :],
                        start=False,
                        stop=True,
                    )
                    stg = st.tile([128, 2, OS], f32, tag="stg")
                    nc.vector.tensor_add(stg[:, 0, :], psm[j], bmean)
                    nc.scalar.activation(out=stg[:, 1, :], in_=psv[j], func=Copy)
                    nc.sync.dma_start(out=out_ap[w * WAVE + j], in_=stg)

    nc.compile()
    _CACHED_NC = nc
    return nc


def _prep_inputs(x, W_logits, b_logits):
    f8np = ml_dtypes.float8_e4m3
    bf16np = ml_dtypes.bfloat16
    xt_8 = np.ascontiguousarray(x.T).astype(f8np)
    xxt_8 = (xt_8.astype(np.float32) ** 2).astype(f8np)
    # logit diffs (softmax is shift invariant): l0-l1, l2-l1
    wdiff = np.stack([W_logits[0] - W_logits[1], W_logits[2] - W_logits[1]])
    bdiff = np.stack(
        [b_logits[0, :, 0] - b_logits[1, :, 0], b_logits[2, :, 0] - b_logits[1, :, 0]]
    ).astype(np.float32)
    in_maps = []
    for c in range(NCORES):
        sl = slice(c * OS, (c + 1) * OS)
        wd_c = np.ascontiguousarray(
            wdiff[:, sl, :].transpose(0, 2, 1)
        ).astype(bf16np)
        bd_c = np.ascontiguousarray(bdiff[:, sl])
        in_maps.append({"xt": xt_8, "xxt": xxt_8, "wd": wd_c, "bd": bd_c})
    return in_maps


def kernel(x, W_logits, b_logits):
    from concourse import bass_utils

    nc = _build()
    in_maps = _prep_inputs(x, W_logits, b_logits)
    res = bass_utils.run_bass_kernel_spmd(
        nc, in_maps, core_ids=list(range(NCORES))
    )
    full = np.empty((N, 2, OUT), dtype=np.float32)
    for c in range(NCORES):
        full[:, :, c * OS : (c + 1) * OS] = res.results[c]["out"]
    return full
